# revision 19
# baseline (speedup 1.0000x reference)
"""Trainium2 Bass kernel for nn_EnhancedAdaptiveMDM (BatchNorm + adaptive
multi-scale pooling pyramid + per-scale gelu MLP residual chain).

Sharding: feature-parallel across 8 cores. Core c owns features
[16c, 16c+16) of F=128. BatchNorm batch-stats are core-local; the only
cross-core exchange is a 37 KB AllGather of per-(b, f) pooled-stat
triples feeding the tiny sel-MLP.

Per-core row layout: r = b*16 + f_local (b-major), 8 row-tiles of 128.
Partition p of row-tile t holds (b = 8t + p//16, f = p%16).

v2 pipeline:
 - BN stats via indicator matmuls into a [128, 256] PSUM layout
   (partition q = 16j + f, column chunk j) so the finalize chain
   (recip/sqrt/etc) runs on all 128 partitions.
 - scale/shift broadcast to [128, S] via per-chunk indicator matmuls on
   the PE (no DMA round trips).
 - normalize with bf16 scale/shift (DVE 2x mode), pairwise pool pyramid,
   per-row pfs stats; per-tile stats shipped to the AllGather bounce
   with one DMA per row-tile.
 - d = avg - max precompute overlapped under the AllGather.
 - sel-MLP on PE; w routed on-chip (PE transpose + indicator matmul)
   to per-partition columns; fused combine stt per (scale, tile),
   DMA-transpose per (scale, tile), lin chain starts on scale 0 while
   later scales still combine.
 - lin chain weights stationary bf16, fp32 PSUM; scale-2 second layer
   emits in natural row layout (bias via ones-row rank-1 matmul), final
   residual add fused reading PSUM + xn, bf16 store (host upcasts).
"""
import os
import sys

import numpy as np

sys.path.insert(0, "/opt/trn_rl_repo")

import ml_dtypes

import concourse.bass as bass
import concourse.bacc as bacc
import concourse.tile as tile
from concourse import mybir
from concourse.bass_utils import run_bass_kernel_spmd

DT = mybir.dt
AL = mybir.AluOpType
AF = mybir.ActivationFunctionType

N_CORES = 8
B, F, S = 64, 128, 2048
FL = F // N_CORES          # 16 local features
R = B * FL                 # 1024 rows per core
T = R // 128               # 8 row-tiles
K_LIST = [8, 4, 2]
L_LIST = [S // k for k in K_LIST]   # 256, 512, 1024
EPS = 1e-5

_CACHE = {}


def _ev(ap):
    return ap.rearrange("p (l two) -> p l two", two=2)[:, :, 0]


def _od(ap):
    return ap.rearrange("p (l two) -> p l two", two=2)[:, :, 1]


def build_nc(single_core_sim=False):
    nc = bacc.Bacc("TRN2", target_bir_lowering=False, debug=False,
                   num_devices=1 if single_core_sim else N_CORES)

    d = {}
    d["x"] = nc.dram_tensor("x", [R, S], DT.bfloat16, kind="ExternalInput").ap()
    d["gamma"] = nc.dram_tensor("gamma", [128, 512], DT.float32, kind="ExternalInput").ap()
    d["beta"] = nc.dram_tensor("beta", [128, 512], DT.float32, kind="ExternalInput").ap()
    d["bnmask"] = nc.dram_tensor("bnmask", [128, 4, 128], DT.float32, kind="ExternalInput").ap()
    d["bcastmask"] = nc.dram_tensor("bcastmask", [128, 4, 128], DT.float32, kind="ExternalInput").ap()
    d["bcmt"] = nc.dram_tensor("bcmt", [64, 8, 128], DT.float32, kind="ExternalInput").ap()
    d["ones1"] = nc.dram_tensor("ones1", [1, 128], DT.float32, kind="ExternalInput").ap()
    d["eye3"] = nc.dram_tensor("eye3", [3, 3], DT.float32, kind="ExternalInput").ap()
    d["eye128"] = nc.dram_tensor("eye128", [128, 128], DT.bfloat16, kind="ExternalInput").ap()
    for s in range(3):
        L = L_LIST[s]
        d[f"w1g_{s}"] = nc.dram_tensor(f"w1g_{s}", [128, 3, 64], DT.float32, kind="ExternalInput").ap()
        d[f"sb1_{s}"] = nc.dram_tensor(f"sb1_{s}", [64], DT.float32, kind="ExternalInput").ap()
        d[f"w2c_{s}"] = nc.dram_tensor(f"w2c_{s}", [64], DT.float32, kind="ExternalInput").ap()
        d[f"sb2_{s}"] = nc.dram_tensor(f"sb2_{s}", [1], DT.float32, kind="ExternalInput").ap()
        d[f"lw1_{s}"] = nc.dram_tensor(f"lw1_{s}", [L, L], DT.bfloat16, kind="ExternalInput").ap()
        d[f"lw2_{s}"] = nc.dram_tensor(f"lw2_{s}", [L, 2 * L], DT.bfloat16, kind="ExternalInput").ap()
        d[f"lb1_{s}"] = nc.dram_tensor(f"lb1_{s}", [L], DT.float32, kind="ExternalInput").ap()
        d[f"lb2_{s}"] = nc.dram_tensor(f"lb2_{s}", [2 * L], DT.float32, kind="ExternalInput").ap()
    d["out"] = nc.dram_tensor("out", [R, S], DT.bfloat16, kind="ExternalOutput").ap()
    if os.environ.get("KDEBUG", "0") == "1":
        d["dbg_xn"] = nc.dram_tensor("dbg_xn", [R, S], DT.bfloat16, kind="ExternalOutput").ap()
        d["dbg_stats"] = nc.dram_tensor("dbg_stats", [128, T * 9], DT.float32, kind="ExternalOutput").ap()
        d["dbg_bounce"] = nc.dram_tensor("dbg_bounce", [16, 576], DT.float32, kind="ExternalOutput").ap()
        d["dbg_w"] = nc.dram_tensor("dbg_w", [1, 192], DT.float32, kind="ExternalOutput").ap()
        d["dbg_wcols"] = nc.dram_tensor("dbg_wcols", [128, T * 3], DT.float32, kind="ExternalOutput").ap()
        d["dbg_scale"] = nc.dram_tensor("dbg_scale", [128, S], DT.bfloat16, kind="ExternalOutput").ap()
        d["dbg_shift"] = nc.dram_tensor("dbg_shift", [128, S], DT.bfloat16, kind="ExternalOutput").ap()
        for s_ in range(3):
            d[f"dbg_sT{s_}"] = nc.dram_tensor(
                f"dbg_sT{s_}", [128, L_LIST[s_] // 128, R], DT.bfloat16, kind="ExternalOutput").ap()

    reps = int(os.environ.get("KAMP", "1"))
    with tile.TileContext(nc) as tc:
        for _ in range(reps):
            _build_body(nc, tc, d, single_core_sim=single_core_sim)
    nc.compile()
    return nc


def _build_body(nc, tc, d, single_core_sim=False):
    perm_cm = tc.tile_pool(name="perm", bufs=1)
    perm = perm_cm.__enter__()
    dram_cm = tc.tile_pool(name="dram", bufs=1, space="DRAM")
    dram = dram_cm.__enter__()
    sT_cm = tc.tile_pool(name="sTall", bufs=1)
    sTp = sT_cm.__enter__()
    w01_cm = tc.tile_pool(name="w01", bufs=1)
    w01 = w01_cm.__enter__()
    p2_cm = tc.tile_pool(name="p2", bufs=1)
    p2 = p2_cm.__enter__()

    # ---- persistent tiles
    xn = perm.tile([128, T, S], DT.bfloat16, tag="xn")  # raw x -> x_norm (bf16)
    scaleB = perm.tile([128, S], DT.bfloat16, tag="scaleB")
    shiftB = perm.tile([128, S], DT.bfloat16, tag="shiftB")
    statsT = perm.tile([128, T, 9], DT.float32, tag="statsT")
    wcols = perm.tile([128, T, 3], DT.float32, tag="wcols")

    sT = [sTp.tile([128, L_LIST[s] // 128, R], DT.bfloat16, name=f"sT{s}")
          for s in range(3)]

    # p2: pool pyramids (sums later become samples in place)
    psum2 = p2.tile([128, T, 1024], DT.bfloat16, tag="psum2")
    pmax2 = p2.tile([128, T, 1024], DT.bfloat16, tag="pmax2")
    psum4 = p2.tile([128, T, 512], DT.bfloat16, tag="psum4")
    pmax4 = p2.tile([128, T, 512], DT.bfloat16, tag="pmax4")
    psum8 = p2.tile([128, T, 256], DT.bfloat16, tag="psum8")
    pmax8 = p2.tile([128, T, 256], DT.bfloat16, tag="pmax8")
    SUMS = {8: psum8, 4: psum4, 2: psum2}
    MAXS = {8: pmax8, 4: pmax4, 2: pmax2}

    # s0/s1 lin weight tiles (DMAs emitted after the x loads)
    wsb = {}
    for s in range(2):
        L = L_LIST[s]
        J = L // 128
        wsb[s] = (
            w01.tile([128, J, L], DT.bfloat16, tag=f"w1sb{s}", name=f"w1sb{s}"),
            w01.tile([128, J, 2 * L], DT.bfloat16, tag=f"w2sb{s}", name=f"w2sb{s}"),
            w01.tile([128, J], DT.float32, tag=f"b1sb{s}", name=f"b1sb{s}"),
            w01.tile([128, 2 * J], DT.float32, tag=f"b2sb{s}", name=f"b2sb{s}"),
        )

    # DRAM bounce tiles for the AllGather
    bounce_in = dram.tile([16, 576], DT.float32)
    gathered = dram.tile([128, 576], DT.float32)

    # =================== Phase 1: load + BN stats ===================
    p1_cm = tc.tile_pool(name="p1", bufs=1)
    p1 = p1_cm.__enter__()
    ps_bn_cm = tc.tile_pool(name="ps_bn", bufs=3, space="PSUM")
    ps_bn = ps_bn_cm.__enter__()
    sq_cm = tc.tile_pool(name="sq", bufs=2)
    sqp = sq_cm.__enter__()

    maskf = p1.tile([128, 4, 128], DT.float32)
    nc.sync.dma_start(out=maskf, in_=d["bnmask"])
    maskbf = p1.tile([128, 4, 128], DT.bfloat16)
    nc.vector.tensor_copy(maskbf, maskf)
    bcastf = p1.tile([128, 4, 128], DT.float32)
    nc.sync.dma_start(out=bcastf, in_=d["bcastmask"])
    bcastb = p1.tile([128, 4, 128], DT.bfloat16)
    nc.vector.tensor_copy(bcastb, bcastf)

    # PSUM layout: partition q = 32*j2 + f holds column chunk j2 (512 cols)
    # of channel f; partitions with q%32 >= 16 are zero padding.
    sum_ps = ps_bn.tile([128, 512], DT.float32, tag="sum_ps")
    sumsq_ps = ps_bn.tile([128, 512], DT.float32, tag="sumsq_ps")

    for t in range(T):
        xeng = nc.sync if t % 2 == 0 else nc.scalar
        xeng.dma_start(out=xn[:, t, :], in_=d["x"][t * 128:(t + 1) * 128, :])
        xsq = sqp.tile([128, S], DT.bfloat16, tag="xsq")
        nc.scalar.activation(out=xsq, in_=xn[:, t, :], func=AF.Square)
        for j2 in range(4):
            sl = slice(j2 * 512, (j2 + 1) * 512)
            first = (t == 0 and j2 == 0)
            last = (t == T - 1 and j2 == 3)
            nc.tensor.matmul(sum_ps, maskbf[:, j2, :], xn[:, t, sl],
                             start=first, stop=last)
            nc.tensor.matmul(sumsq_ps, maskbf[:, j2, :], xsq[:, sl],
                             start=first, stop=last)

    # prefetch s0/s1 lin weights behind the x loads
    for s in range(2):
        nc.sync.dma_start(out=wsb[s][0], in_=d[f"lw1_{s}"].rearrange("(j p) n -> p j n", p=128))
        nc.sync.dma_start(out=wsb[s][1], in_=d[f"lw2_{s}"].rearrange("(j p) n -> p j n", p=128))
        nc.sync.dma_start(out=wsb[s][2], in_=d[f"lb1_{s}"].rearrange("(j p) -> p j", p=128))
        nc.sync.dma_start(out=wsb[s][3], in_=d[f"lb2_{s}"].rearrange("(j p) -> p j", p=128))

    # ---- BN finalize on [128, 512] (q = 32*j2 + f)
    mu = p1.tile([128, 512], DT.float32)
    nc.scalar.mul(out=mu, in_=sum_ps, mul=1.0 / B)
    ex2 = p1.tile([128, 512], DT.float32)
    nc.scalar.mul(out=ex2, in_=sumsq_ps, mul=1.0 / B)
    var = p1.tile([128, 512], DT.float32)
    nc.vector.tensor_tensor(out=var, in0=mu, in1=mu, op=AL.mult)       # mu^2
    nc.vector.tensor_tensor(out=var, in0=ex2, in1=var, op=AL.subtract)
    epscol = p1.tile([128, 1], DT.float32)
    nc.vector.memset(epscol, EPS)
    sd = p1.tile([128, 512], DT.float32)
    nc.scalar.activation(out=sd, in_=var, func=AF.Sqrt, bias=epscol)
    nc.vector.reciprocal(out=sd, in_=sd)                               # rstd
    gam = p1.tile([128, 512], DT.float32)
    nc.sync.dma_start(out=gam, in_=d["gamma"])
    nc.vector.tensor_tensor(out=sd, in0=gam, in1=sd, op=AL.mult)       # scale
    nc.sync.dma_start(out=gam, in_=d["beta"])                          # beta
    shf = p1.tile([128, 512], DT.float32)
    nc.vector.tensor_tensor(out=shf, in0=mu, in1=sd, op=AL.mult)       # mu*scale
    nc.vector.tensor_tensor(out=shf, in0=gam, in1=shf, op=AL.subtract)
    sclb = p1.tile([128, 512], DT.bfloat16)
    nc.vector.tensor_copy(sclb, sd)
    shfb = p1.tile([128, 512], DT.bfloat16)
    nc.vector.tensor_copy(shfb, shf)

    # broadcast [128(q=32*j2+f), 512] -> [128(p=16b+f), 2048] via PE
    for j2 in range(4):
        psB = ps_bn.tile([128, 512], DT.float32, tag="psB", bufs=2)
        nc.tensor.matmul(psB, bcastb[:, j2, :], sclb, start=True, stop=True)
        nc.scalar.copy(out=scaleB[:, j2 * 512:(j2 + 1) * 512], in_=psB)
        psB2 = ps_bn.tile([128, 512], DT.float32, tag="psB", bufs=2)
        nc.tensor.matmul(psB2, bcastb[:, j2, :], shfb, start=True, stop=True)
        nc.scalar.copy(out=shiftB[:, j2 * 512:(j2 + 1) * 512], in_=psB2)

    sq_cm.__exit__(None, None, None)
    ps_bn_cm.__exit__(None, None, None)
    p1_cm.__exit__(None, None, None)

    # =================== Phase 2: normalize + pools + stats ===================
    scr_cm = tc.tile_pool(name="scr", bufs=2)
    scr = scr_cm.__enter__()

    with nc.allow_low_precision("bf16 pools within 2e-2 tolerance"):
        for t in range(T):
            xt = xn[:, t, :]
            nc.vector.tensor_tensor(out=xt, in0=xt, in1=scaleB, op=AL.mult)
            nc.vector.tensor_tensor(out=xt, in0=xt, in1=shiftB, op=AL.add)
            nc.vector.tensor_tensor(out=psum2[:, t, :], in0=_ev(xt), in1=_od(xt), op=AL.add)
            nc.vector.tensor_tensor(out=pmax2[:, t, :], in0=_ev(xt), in1=_od(xt), op=AL.max)
            s2 = psum2[:, t, :]
            m2 = pmax2[:, t, :]
            nc.vector.tensor_tensor(out=psum4[:, t, :], in0=_ev(s2), in1=_od(s2), op=AL.add)
            nc.vector.tensor_tensor(out=pmax4[:, t, :], in0=_ev(m2), in1=_od(m2), op=AL.max)
            s4 = psum4[:, t, :]
            m4 = pmax4[:, t, :]
            nc.vector.tensor_tensor(out=psum8[:, t, :], in0=_ev(s4), in1=_od(s4), op=AL.add)
            nc.vector.tensor_tensor(out=pmax8[:, t, :], in0=_ev(m4), in1=_od(m4), op=AL.max)

            # per-scale pfs stats: pfs' = sum/k + mx ; (sum, std, max) of pfs'
            sqcols = scr.tile([128, 3], DT.float32, tag="sqcols")
            for s, k in enumerate(K_LIST):
                L = L_LIST[s]
                pfs_t = scr.tile([128, 1024], DT.bfloat16, tag="pfs")
                pfs = pfs_t[:, :L]
                nc.vector.scalar_tensor_tensor(
                    out=pfs, in0=SUMS[k][:, t, :], scalar=1.0 / k, in1=MAXS[k][:, t, :],
                    op0=AL.mult, op1=AL.add,
                    accum_out=statsT[:, t, 3 * s:3 * s + 1])
                junk_t = scr.tile([128, 1024], DT.bfloat16, tag="junk")
                junk = junk_t[:, :L]
                nc.scalar.activation(out=junk, in_=pfs, func=AF.Square,
                                     accum_out=sqcols[:, s:s + 1])
                nc.vector.tensor_reduce(
                    out=statsT[:, t, 3 * s + 2:3 * s + 3], in_=pfs,
                    axis=mybir.AxisListType.X, op=AL.max)
            # batched var/std across the 3 scales
            sums3 = statsT[:, t, :].rearrange("p (s m) -> p s m", m=3)[:, :, 0]
            sq3 = scr.tile([128, 3], DT.float32, tag="sq3")
            nc.vector.tensor_tensor(out=sq3, in0=sums3, in1=sums3, op=AL.mult)
            for s in range(3):
                L = L_LIST[s]
                vcol = scr.tile([128, 1], DT.float32, tag="vcol")
                nc.vector.scalar_tensor_tensor(
                    out=vcol, in0=sq3[:, s:s + 1], scalar=-1.0 / L,
                    in1=sqcols[:, s:s + 1], op0=AL.mult, op1=AL.add)
                nc.scalar.activation(out=statsT[:, t, 3 * s + 1:3 * s + 2], in_=vcol,
                                     func=AF.Sqrt, scale=1.0 / (L - 1))
            # ship tile stats to the AllGather bounce: col = t*72 + bl*9 + sm
            src = bass.AP(tensor=statsT.tensor,
                          offset=statsT.offset + t * 9,
                          ap=[[T * 9, 128], [1, 9]])
            dst = bass.AP(tensor=bounce_in.tensor,
                          offset=bounce_in.offset + t * 72,
                          ap=[[9, 8], [576, 16], [1, 9]])
            nc.scalar.dma_start(out=dst, in_=src)

        # =================== Phase 3: AllGather + sel MLP ===================
        if single_core_sim or os.environ.get("KAG", "1") == "0":
            for c_ in range(N_CORES):
                nc.sync.dma_start(out=gathered[c_ * 16:(c_ + 1) * 16, :], in_=bounce_in[:])
        else:
            nc.gpsimd.collective_compute(
                "AllGather", AL.bypass,
                ins=[bounce_in.opt()],
                outs=[gathered.opt()],
                replica_groups=[list(range(N_CORES))],
            )

        # d = sum/k - mx precompute, overlapped under the AllGather
        for s, k in enumerate(K_LIST):
            for t in range(T):
                sm = SUMS[k][:, t, :]
                nc.vector.scalar_tensor_tensor(
                    out=sm, in0=sm, scalar=1.0 / k, in1=MAXS[k][:, t, :],
                    op0=AL.mult, op1=AL.subtract)

    if "dbg_xn" in d:
        for t in range(T):
            nc.sync.dma_start(out=d["dbg_xn"][t * 128:(t + 1) * 128, :], in_=xn[:, t, :])
        nc.sync.dma_start(out=d["dbg_stats"], in_=statsT[:].rearrange("p t q -> p (t q)"))
        nc.sync.dma_start(out=d["dbg_bounce"], in_=bounce_in[:])
        nc.sync.dma_start(out=d["dbg_scale"], in_=scaleB)
        nc.sync.dma_start(out=d["dbg_shift"], in_=shiftB)

    mlp_cm = tc.tile_pool(name="mlp", bufs=1)
    mlp = mlp_cm.__enter__()
    ps_mlp_cm = tc.tile_pool(name="ps_mlp", bufs=2, space="PSUM")
    ps_mlp = ps_mlp_cm.__enter__()

    # sel-MLP params: loaded/cast up front (independent of the AllGather)
    bcm_f = mlp.tile([64, 8, 128], DT.float32)
    nc.sync.dma_start(out=bcm_f, in_=d["bcmt"])
    bcmt = mlp.tile([64, 8, 128], DT.bfloat16)
    nc.vector.tensor_copy(bcmt, bcm_f)
    eye3f = mlp.tile([3, 3], DT.float32)
    nc.sync.dma_start(out=eye3f, in_=d["eye3"])
    eye128b = mlp.tile([128, 128], DT.bfloat16)
    nc.sync.dma_start(out=eye128b, in_=d["eye128"])
    w1b = mlp.tile([128, 3, 3, 64], DT.bfloat16)
    w1f = mlp.tile([128, 3, 3, 64], DT.float32)
    for s in range(3):
        nc.sync.dma_start(out=w1f[:, s, :, :], in_=d[f"w1g_{s}"])
    nc.vector.tensor_copy(w1b, w1f)
    b1all = mlp.tile([64, 3], DT.float32)
    w2all = mlp.tile([64, 3], DT.float32)
    b2all = mlp.tile([1, 3], DT.float32)
    for s in range(3):
        nc.sync.dma_start(out=b1all[:, s:s + 1], in_=d[f"sb1_{s}"].rearrange("(n one) -> n one", one=1))
        nc.sync.dma_start(out=w2all[:, s:s + 1], in_=d[f"w2c_{s}"].rearrange("(n one) -> n one", one=1))
        nc.sync.dma_start(out=b2all[:, s:s + 1], in_=d[f"sb2_{s}"].rearrange("(n one) -> n one", one=1))
    w2allb = mlp.tile([64, 3], DT.bfloat16)
    nc.vector.tensor_copy(w2allb, w2all)

    gsb_f = mlp.tile([128, 576], DT.float32)
    nc.sync.dma_start(out=gsb_f, in_=gathered[:])
    gsb = mlp.tile([128, 576], DT.bfloat16)
    nc.vector.tensor_copy(gsb, gsb_f)

    # all 3 scales batched: psU [64, 3, 64] -> one relu -> uT [64, 3, 64]
    wrows = mlp.tile([1, 3, 64], DT.float32)
    psT = ps_mlp.tile([64, 3], DT.float32, tag="psT")
    psU = ps_mlp.tile([64, 3, 64], DT.float32, tag="psU")
    gsbv = gsb.rearrange("p (t bl sm) -> p t bl sm", bl=8, sm=9)
    for s in range(3):
        for m in range(3):
            nc.tensor.matmul(psU[:, s, :], w1b[:, s, m, :], gsbv[:, :, :, 3 * s + m],
                             start=(m == 0), stop=(m == 2))
    uT = mlp.tile([64, 3, 64], DT.bfloat16)
    for s in range(3):
        nc.scalar.activation(out=uT[:, s, :], in_=psU[:, s, :],
                             func=AF.Relu, bias=b1all[:, s:s + 1])
    psW = ps_mlp.tile([1, 3, 64], DT.float32, tag="psW")
    for s in range(3):
        nc.tensor.matmul(psW[:, s, :], w2allb[:, s:s + 1], uT[:, s, :],
                         start=True, stop=True)
    for s in range(3):
        nc.scalar.activation(out=wrows[:, s, :], in_=psW[:, s, :],
                             func=AF.Sigmoid, bias=b2all[:, s:s + 1])
        nc.tensor.transpose(psT[:, s:s + 1], wrows[:, s, :], eye3f[0:1, 0:1])

    # w routing on-chip: [64, 3] -> per-tile [128, 3] via indicator matmul
    wT = mlp.tile([64, 3], DT.float32)
    nc.scalar.copy(out=wT, in_=psT)
    wTb = mlp.tile([64, 3], DT.bfloat16)
    nc.vector.tensor_copy(wTb, wT)
    for t in range(T):
        psWC = ps_mlp.tile([128, 3], DT.float32, tag="psWC", bufs=2)
        nc.tensor.matmul(psWC, bcmt[:, t, :], wTb, start=True, stop=True)
        nc.scalar.copy(out=wcols[:, t, :], in_=psWC)

    if "dbg_w" in d:
        nc.sync.dma_start(out=d["dbg_w"], in_=wrows[:].rearrange("p a b -> p (a b)"))
        nc.sync.dma_start(out=d["dbg_wcols"], in_=wcols[:].rearrange("p t q -> p (t q)"))

    # =================== Phase 4: combine + transpose (scale 0 first) ========
    for s, k in enumerate(K_LIST):
        for t in range(T):
            sm = SUMS[k][:, t, :]
            nc.vector.scalar_tensor_tensor(
                out=sm, in0=sm, scalar=wcols[:, t, s:s + 1], in1=MAXS[k][:, t, :],
                op0=AL.mult, op1=AL.add)
            nc.sync.dma_start_transpose(sT[s][:, :, t * 128:(t + 1) * 128], sm)

    if "dbg_sT0" in d:
        for s_ in range(3):
            nc.sync.dma_start(out=d[f"dbg_sT{s_}"], in_=sT[s_])

    ps_mlp_cm.__exit__(None, None, None)
    mlp_cm.__exit__(None, None, None)
    scr_cm.__exit__(None, None, None)
    p2_cm.__exit__(None, None, None)

    # =================== Phase 5: lin chain ===================
    wp2_cm = tc.tile_pool(name="wp2", bufs=1)
    wp2 = wp2_cm.__enter__()
    hpool_cm = tc.tile_pool(name="hpool", bufs=1)
    hpool = hpool_cm.__enter__()
    fin_cm = tc.tile_pool(name="fin", bufs=2)
    fin_pool = fin_cm.__enter__()
    ps_lin_cm = tc.tile_pool(name="ps_lin", bufs=4, space="PSUM")
    ps_lin = ps_lin_cm.__enter__()
    ps_nat_cm = tc.tile_pool(name="ps_nat", bufs=4, space="PSUM")
    ps_nat = ps_nat_cm.__enter__()

    # scale-2 weights (after pool memory is released)
    L2 = L_LIST[2]
    w1sb2 = wp2.tile([128, 8, L2], DT.bfloat16, tag="w1sb2")
    nc.sync.dma_start(out=w1sb2, in_=d["lw1_2"].rearrange("(j p) n -> p j n", p=128))
    w2sb2 = wp2.tile([128, 8, 2 * L2], DT.bfloat16, tag="w2sb2")
    nc.sync.dma_start(out=w2sb2, in_=d["lw2_2"].rearrange("(j p) n -> p j n", p=128))
    b1sb2 = wp2.tile([128, 8], DT.float32, tag="b1sb2")
    nc.sync.dma_start(out=b1sb2, in_=d["lb1_2"].rearrange("(j p) -> p j", p=128))
    b2rf = wp2.tile([1, 2 * L2], DT.float32, tag="b2rf")
    nc.sync.dma_start(out=b2rf, in_=d["lb2_2"].rearrange("(one n) -> one n", one=1))
    b2row = wp2.tile([1, 2 * L2], DT.bfloat16, tag="b2row")
    nc.vector.tensor_copy(b2row, b2rf)
    onesf = wp2.tile([1, 128], DT.float32, tag="onesf")
    nc.sync.dma_start(out=onesf, in_=d["ones1"])
    ones1b = wp2.tile([1, 128], DT.bfloat16, tag="ones1b")
    nc.vector.tensor_copy(ones1b, onesf)

    for s in range(3):
        L = L_LIST[s]
        J = L // 128
        if s < 2:
            w1sb, w2sb, b1sb, b2sb = wsb[s]
        else:
            w1sb, w2sb, b1sb = w1sb2, w2sb2, b1sb2

        hT = hpool.tile([128, J, R], DT.bfloat16, tag=f"hT{s}")
        for mi in range(J):
            psH = [ps_lin.tile([128, 512], DT.float32, tag="psH", bufs=2,
                               name=f"psH{s}_{mi}_{h}") for h in range(2)]
            for ki in range(J):
                for h in range(2):
                    nc.tensor.matmul(
                        psH[h], w1sb[:, ki, mi * 128:(mi + 1) * 128],
                        sT[s][:, ki, h * 512:(h + 1) * 512],
                        start=(ki == 0), stop=(ki == J - 1))
            for h in range(2):
                nc.scalar.activation(out=hT[:, mi, h * 512:(h + 1) * 512], in_=psH[h],
                                     func=AF.Gelu, bias=b1sb[:, mi:mi + 1])
        if s < 2:
            for m2 in range(2 * J):
                psO = [ps_lin.tile([128, 512], DT.float32, tag="psO", bufs=2,
                                   name=f"psO{s}_{m2}_{h}") for h in range(2)]
                for ki in range(J):
                    for h in range(2):
                        nc.tensor.matmul(
                            psO[h], w2sb[:, ki, m2 * 128:(m2 + 1) * 128],
                            hT[:, ki, h * 512:(h + 1) * 512],
                            start=(ki == 0), stop=(ki == J - 1))
                for h in range(2):
                    dst = sT[s + 1][:, m2, h * 512:(h + 1) * 512]
                    nc.vector.scalar_tensor_tensor(
                        out=dst, in0=psO[h], scalar=b2sb[:, m2:m2 + 1], in1=dst,
                        op0=AL.add, op1=AL.add)
        else:
            # natural-layout second layer + fused bias/residual/store
            for t in range(T):
                psN = [ps_nat.tile([128, 512], DT.float32, tag=f"psN{c4}", bufs=1,
                                   name=f"psN{t}_{c4}") for c4 in range(4)]
                for c4 in range(4):
                    nc.tensor.matmul(psN[c4], ones1b,
                                     b2row[:, c4 * 512:(c4 + 1) * 512],
                                     start=True, stop=False)
                for ki in range(J):
                    for c4 in range(4):
                        nc.tensor.matmul(
                            psN[c4], hT[:, ki, t * 128:(t + 1) * 128],
                            w2sb[:, ki, c4 * 512:(c4 + 1) * 512],
                            start=False, stop=(ki == J - 1))
                fin = fin_pool.tile([128, S], DT.bfloat16, tag="fin", name=f"fin{t}")
                for c4 in range(4):
                    nc.vector.tensor_tensor(
                        out=fin[:, c4 * 512:(c4 + 1) * 512], in0=psN[c4],
                        in1=xn[:, t, c4 * 512:(c4 + 1) * 512], op=AL.add)
                nc.sync.dma_start(out=d["out"][t * 128:(t + 1) * 128, :], in_=fin)

    ps_nat_cm.__exit__(None, None, None)
    ps_lin_cm.__exit__(None, None, None)
    fin_cm.__exit__(None, None, None)
    hpool_cm.__exit__(None, None, None)
    wp2_cm.__exit__(None, None, None)
    w01_cm.__exit__(None, None, None)
    sT_cm.__exit__(None, None, None)
    dram_cm.__exit__(None, None, None)
    perm_cm.__exit__(None, None, None)


def _host_prep(inputs):
    """Build the 8 per-core in_maps from full inputs."""
    x = np.asarray(inputs["x"], np.float32)
    g2 = np.asarray(inputs["bn_gamma"], np.float32).reshape(F, S)
    b2 = np.asarray(inputs["bn_beta"], np.float32).reshape(F, S)

    bnmask = np.zeros((128, 4, 128), np.float32)
    for j2 in range(4):
        for p in range(128):
            bnmask[p, j2, 32 * j2 + (p % 16)] = 1.0

    bcastmask = np.zeros((128, 4, 128), np.float32)
    for j2 in range(4):
        for q in range(128):
            bcastmask[32 * j2 + (q % 16), j2, q] = 1.0

    bcmt = np.zeros((64, 8, 128), np.float32)
    for t in range(8):
        for q in range(128):
            bcmt[8 * t + q // 16, t, q] = 1.0

    common = {"bnmask": bnmask, "bcastmask": bcastmask, "bcmt": bcmt,
              "ones1": np.ones((1, 128), np.float32),
              "eye3": np.eye(3, dtype=np.float32),
              "eye128": np.eye(128, dtype=np.float32).astype(ml_dtypes.bfloat16)}
    for s, k in enumerate(K_LIST):
        L = L_LIST[s]
        w1 = np.asarray(inputs[f"selw1_{s}"], np.float32)      # [3F, 64]
        sc = np.array([0.5 / L, 0.5, 0.5], np.float32)
        # w1g[p, m, :] = w1[m*F + p] * sc[m]
        w1g = np.ascontiguousarray(
            w1.reshape(3, F, 64).transpose(1, 0, 2) * sc[None, :, None])
        common[f"w1g_{s}"] = w1g
        common[f"sb1_{s}"] = np.asarray(inputs[f"selb1_{s}"], np.float32).reshape(64)
        common[f"w2c_{s}"] = np.asarray(inputs[f"selw2_{s}"], np.float32).reshape(64)
        common[f"sb2_{s}"] = np.asarray(inputs[f"selb2_{s}"], np.float32).reshape(1)
        common[f"lw1_{s}"] = np.asarray(inputs[f"linw1_{s}"], np.float32).astype(ml_dtypes.bfloat16)
        common[f"lw2_{s}"] = np.asarray(inputs[f"linw2_{s}"], np.float32).astype(ml_dtypes.bfloat16)
        common[f"lb1_{s}"] = np.asarray(inputs[f"linb1_{s}"], np.float32).reshape(L)
        common[f"lb2_{s}"] = np.asarray(inputs[f"linb2_{s}"], np.float32).reshape(2 * L)

    in_maps = []
    for c in range(N_CORES):
        m = dict(common)
        m["x"] = np.ascontiguousarray(
            x[:, c * FL:(c + 1) * FL, :]).reshape(R, S).astype(ml_dtypes.bfloat16)
        # gamma/beta in [128, 512] layout: row 32*j2 + f, col chunk j2
        gl = g2[c * FL:(c + 1) * FL].reshape(FL, 4, 512)
        ga = np.zeros((128, 512), np.float32)
        bl = b2[c * FL:(c + 1) * FL].reshape(FL, 4, 512)
        be = np.zeros((128, 512), np.float32)
        for j2 in range(4):
            ga[32 * j2:32 * j2 + FL] = gl[:, j2]
            be[32 * j2:32 * j2 + FL] = bl[:, j2]
        m["gamma"] = ga
        m["beta"] = be
        in_maps.append(m)
    return in_maps


def kernel(**inputs) -> np.ndarray:
    if "nc" not in _CACHE:
        _CACHE["nc"] = build_nc()
    nc = _CACHE["nc"]
    in_maps = _host_prep(inputs)
    trace = os.environ.get("BASS_KERNEL_TRACE", "0") == "1"
    res = run_bass_kernel_spmd(nc, in_maps, core_ids=list(range(N_CORES)),
                               trace=trace)
    _CACHE["last_result"] = res
    out = np.empty((B, F, S), np.float32)
    for c in range(N_CORES):
        oc = res.results[c]["out"].astype(np.float32).reshape(B, FL, S)
        out[:, c * FL:(c + 1) * FL, :] = oc
    return out
